# revision 12
# baseline (speedup 1.0000x reference)
"""BitLinear forward kernel for Trainium2 (8 NeuronCores, data-parallel).

Computes y = sign(x) @ (alpha * code)^T + b where code/alpha are the
per-row ternarization of W (BitNet-style, delta_w = 0.05, delta_a = 0.0).

Sharding: x is split over batch*seq (16384 rows) across 8 cores; W is
replicated (each core quantizes the full W on-device); outputs are
concatenated on the host.

The matmul runs in fp8 DoubleRow mode (operand values are exactly
{-1, 0, +1}; two d-tiles are paired per PE pass) with fp32 PSUM
accumulation, so integer counts are exact; the per-output-feature alpha
scale is applied in fp32 on eviction. b from setup_inputs() is zeros; a
nonzero b takes a second elementwise pass.

Layout: the contraction dim (d) must live on SBUF partitions for both
matmul operands, so sign(x) and code are bounced through DRAM (bf16) in
quarter-row blocks and read back through the DMA xbar transpose, then
cast to fp8 into paired-d-tile operands [128, 2, 512].

alpha identity: sum(aWc * (aWc>=thr)) == sum(relu(aWc-thr)) + thr*count,
so the numerator comes from an ACT Relu pass with accumulate.
"""

import sys

for _p in ("/opt/trn_rl_repo", "/opt/trn_rl_repo/concourse"):
    if _p not in sys.path:
        sys.path.insert(0, _p)

import numpy as np

import concourse.bass as bass
import concourse.tile as tile
import concourse.mybir as mybir
from concourse import bacc
from concourse.bass_utils import run_bass_kernel_spmd

# Problem shape (hardcoded per contract)
B, S, D, O = 4, 4096, 2048, 2048
N_CORES = 8
T = (B * S) // N_CORES  # 2048 token rows per core
DELTA_W = 0.05

P = 128
TT = T // P   # 16 t-tiles
DT = D // P   # 16 d-tiles
DP = DT // 2  # 8 d-pair tiles (DoubleRow)
WT = O // P   # 16 W row-tiles
NB = 4        # psum banks per t-tile (512 f32 each)
NBW = O // NB  # 512
Q = 4         # quarter blocks (o-banks and t-quarters)
QR = T // Q   # 512 rows per quarter

F32 = mybir.dt.float32
BF16 = mybir.dt.bfloat16
FP8 = mybir.dt.float8e4
U16 = mybir.dt.uint16

_CACHE = {}


def _build(with_bias: bool):
    nc = bacc.Bacc("TRN2", target_bir_lowering=False, debug=False,
                   num_devices=N_CORES)
    x_d = nc.dram_tensor("x", [T, D], F32, kind="ExternalInput").ap()
    w_d = nc.dram_tensor("W", [O, D], F32, kind="ExternalInput").ap()
    y_d = nc.dram_tensor("y", [T, O], F32, kind="ExternalOutput").ap()
    if with_bias:
        b_d = nc.dram_tensor("b", [O], F32, kind="ExternalInput").ap()

    with tile.TileContext(nc) as tc:
        with (
            tc.tile_pool(name="dram", bufs=16, space="DRAM") as dram,
            tc.tile_pool(name="wload", bufs=2) as wload,
            tc.tile_pool(name="awc", bufs=2) as awc_pool,
            tc.tile_pool(name="junk", bufs=1) as junk_pool,
            tc.tile_pool(name="wsmall", bufs=2) as wsmall,
            tc.tile_pool(name="stats", bufs=1) as stats,
            tc.tile_pool(name="xload", bufs=2) as xload,
            tc.tile_pool(name="xsign", bufs=2) as xsign,
            tc.tile_pool(name="tpose", bufs=4) as tpose,
            tc.tile_pool(name="codeT", bufs=DP * NB) as codeT_pool,
            tc.tile_pool(name="xqT", bufs=DP * Q) as xqT_pool,
            tc.tile_pool(name="psum", bufs=2, space="PSUM") as psum_pool,
            tc.tile_pool(name="yout", bufs=2) as yout,
            tc.tile_pool(name="bcast", bufs=1) as bcast,
        ):
            # quarter-row DRAM bounce tiles (fine dependency granularity)
            xq_dram = [dram.tile([QR, D], BF16, tag=f"xqd{g}",
                                 name=f"xq_dram{g}") for g in range(Q)]
            code_dram = [dram.tile([QR, D], BF16, tag=f"cdd{g}",
                                   name=f"code_dram{g}") for g in range(Q)]
            alpha_dram = dram.tile([O], F32, tag="alphad")

            # Per-row stats, one column per W row-tile
            S_all = stats.tile([P, WT], F32, tag="S")
            T_all = stats.tile([P, WT], F32, tag="T")
            den_all = stats.tile([P, WT], F32, tag="den")
            relu_all = stats.tile([P, WT], F32, tag="relu")
            negmean_all = stats.tile([P, WT], F32, tag="negmean")
            thr_all = stats.tile([P, WT], F32, tag="thr")
            negthr_all = stats.tile([P, WT], F32, tag="negthr")
            alpha_all = stats.tile([P, WT], F32, tag="alpha")

            act_junk = junk_pool.tile([P, D], F32, tag="act_junk")

            # pair tiles: [128, 2*512] fp8; halves = d-tiles (2dp, 2dp+1)
            codeT8 = [[None] * NB for _ in range(DP)]
            xqT8 = [[None] * Q for _ in range(DP)]
            for dp in range(DP):
                for o4 in range(NB):
                    codeT8[dp][o4] = codeT_pool.tile(
                        [P, 2 * NBW], FP8, tag="codeT",
                        name=f"codeT{dp}_{o4}")
                for q in range(Q):
                    xqT8[dp][q] = xqT_pool.tile(
                        [P, 2 * QR], FP8, tag="xqT", name=f"xqT{dp}_{q}")

            for g in range(Q):
                # ---- quarter g of the W pipeline (wi = 4g..4g+3) --------
                for j in range(Q):
                    wi = g * Q + j
                    wt = wload.tile([P, D], F32)
                    nc.gpsimd.dma_start(wt[:], w_d[wi * P:(wi + 1) * P, :])
                    # S = sum(W) via ACT Copy with accumulate
                    nc.scalar.activation(
                        out=act_junk[:], in_=wt[:],
                        func=mybir.ActivationFunctionType.Copy,
                        accum_out=S_all[:, wi:wi + 1],
                    )
                    nc.vector.tensor_scalar_mul(
                        negmean_all[:, wi:wi + 1], S_all[:, wi:wi + 1],
                        -1.0 / D,
                    )
                    # aWc = |W - mean|, T = sum(aWc)
                    aWc = awc_pool.tile([P, D], F32)
                    nc.scalar.activation(
                        out=aWc[:], in_=wt[:],
                        func=mybir.ActivationFunctionType.Abs,
                        bias=negmean_all[:, wi:wi + 1],
                        accum_out=T_all[:, wi:wi + 1],
                    )
                    # thr = DELTA_W/D * T ; negthr = -thr (from T directly)
                    nc.vector.tensor_scalar_mul(
                        thr_all[:, wi:wi + 1], T_all[:, wi:wi + 1],
                        DELTA_W / D,
                    )
                    nc.vector.tensor_scalar_mul(
                        negthr_all[:, wi:wi + 1], T_all[:, wi:wi + 1],
                        -DELTA_W / D,
                    )
                    # sgn = Sign(W - mean) bf16
                    sgn = wsmall.tile([P, D], BF16, tag="sgn")
                    nc.scalar.activation(
                        out=sgn[:], in_=wt[:],
                        func=mybir.ActivationFunctionType.Sign,
                        bias=negmean_all[:, wi:wi + 1],
                    )
                    # R = sum(relu(aWc - thr))  (alpha numerator part)
                    nc.scalar.activation(
                        out=act_junk[:], in_=aWc[:],
                        func=mybir.ActivationFunctionType.Relu,
                        bias=negthr_all[:, wi:wi + 1],
                        accum_out=relu_all[:, wi:wi + 1],
                    )
                    # s01 = (aWc >= thr), den = count
                    s01 = wsmall.tile([P, D], BF16, tag="s01")
                    nc.vector.tensor_scalar(
                        out=s01[:], in0=aWc[:],
                        scalar1=thr_all[:, wi:wi + 1], scalar2=0.0,
                        op0=mybir.AluOpType.is_ge,
                        op1=mybir.AluOpType.add,
                        accum_out=den_all[:, wi:wi + 1],
                    )
                    # code = sgn * s01 (gpsimd; values exactly -1/0/1)
                    code = wsmall.tile([P, D], BF16, tag="code")
                    nc.gpsimd.tensor_mul(code[:], sgn[:], s01[:])
                    nc.scalar.dma_start(
                        code_dram[g][j * P:(j + 1) * P, :], code[:])

                    # ---- same-index t-tile of the x pipeline ------------
                    ti = g * Q + j
                    xb = xload.tile([P, D], BF16)
                    nc.gpsimd.dma_start(
                        xb[:], x_d[ti * P:(ti + 1) * P, :])  # f32->bf16
                    xq = xsign.tile([P, D], BF16)
                    nc.vector.tensor_scalar(
                        out=xq.bitcast(U16)[:], in0=xb.bitcast(U16)[:],
                        scalar1=0x8000, scalar2=0x3F80,
                        op0=mybir.AluOpType.bitwise_and,
                        op1=mybir.AluOpType.bitwise_or,
                    )
                    nc.scalar.dma_start(
                        xq_dram[g][j * P:(j + 1) * P, :], xq[:])

                # ---- quarter g transposed reads + fp8 pair converts -----
                for di in range(DT):
                    dp, half = divmod(di, 2)
                    tb = tpose.tile([P, NBW], BF16, tag="tp_code")
                    nc.sync.dma_start_transpose(
                        tb[:], code_dram[g][:, di * P:(di + 1) * P])
                    nc.vector.tensor_copy(
                        out=codeT8[dp][g][:, half * NBW:(half + 1) * NBW],
                        in_=tb[:])
                    tb2 = tpose.tile([P, QR], BF16, tag="tp_xq")
                    nc.sync.dma_start_transpose(
                        tb2[:], xq_dram[g][:, di * P:(di + 1) * P])
                    nc.gpsimd.tensor_copy(
                        out=xqT8[dp][g][:, half * QR:(half + 1) * QR],
                        in_=tb2[:])

            # ------ alpha = (R + thr*den) / max(den, 1) ------------------
            num = stats.tile([P, WT], F32, tag="num")
            nc.vector.tensor_mul(num[:], thr_all[:], den_all[:])
            nc.vector.tensor_add(num[:], num[:], relu_all[:])
            denc = stats.tile([P, WT], F32, tag="denc")
            nc.vector.tensor_scalar_max(denc[:], den_all[:], 1.0)
            rden = stats.tile([P, WT], F32, tag="rden")
            nc.vector.reciprocal(rden[:], denc[:])
            nc.vector.tensor_mul(alpha_all[:], num[:], rden[:])
            nc.sync.dma_start(
                alpha_dram.rearrange("(w p) -> p w", p=P)[:, :], alpha_all[:])
            alphaB = bcast.tile([P, O], F32, tag="alphaB")
            nc.gpsimd.dma_start(
                alphaB[:], alpha_dram.unsqueeze(0).to_broadcast((P, O)))
            if with_bias:
                biasB = bcast.tile([P, O], F32, tag="biasB")
                nc.gpsimd.dma_start(
                    biasB[:], b_d.unsqueeze(0).to_broadcast((P, O)))

            # ---------------- main matmul (DoubleRow fp8) ----------------
            for ti in range(TT):
                ps = psum_pool.tile([P, O], F32)
                q = ti // (TT // Q)
                r = ti % (TT // Q)
                for dp in range(DP):
                    lhsT = xqT8[dp][q].rearrange(
                        "p (two m) -> p two m", two=2)[:, :, r * P:(r + 1) * P]
                    for o4 in range(NB):
                        rhs = codeT8[dp][o4].rearrange(
                            "p (two n) -> p two n", two=2)
                        nc.tensor.matmul(
                            ps[:, o4 * NBW:(o4 + 1) * NBW],
                            lhsT,
                            rhs,
                            start=(dp == 0),
                            stop=(dp == DP - 1),
                            perf_mode=mybir.MatmulPerfMode.DoubleRow,
                        )
                ysb = yout.tile([P, O], F32)
                nc.vector.tensor_mul(ysb[:], ps[:], alphaB[:])
                if with_bias:
                    nc.vector.tensor_add(ysb[:], ysb[:], biasB[:])
                nc.scalar.dma_start(y_d[ti * P:(ti + 1) * P, :], ysb[:])

    nc.compile()
    return nc


def _get_nc(with_bias: bool):
    key = with_bias
    if key not in _CACHE:
        _CACHE[key] = _build(with_bias)
    return _CACHE[key]


def kernel(x: np.ndarray, W: np.ndarray, b: np.ndarray) -> np.ndarray:
    x = np.asarray(x, dtype=np.float32)
    W = np.ascontiguousarray(W, dtype=np.float32)
    b = np.asarray(b, dtype=np.float32)
    with_bias = bool(np.any(b))

    nc = _get_nc(with_bias)

    xf = np.ascontiguousarray(x.reshape(B * S, D))
    in_maps = []
    for c in range(N_CORES):
        m = {"x": np.ascontiguousarray(xf[c * T:(c + 1) * T]), "W": W}
        if with_bias:
            m["b"] = b
        in_maps.append(m)

    res = run_bass_kernel_spmd(nc, in_maps, core_ids=list(range(N_CORES)))
    y = np.concatenate([res.results[c]["y"] for c in range(N_CORES)], axis=0)
    return np.ascontiguousarray(y.reshape(B, S, O))


if __name__ == "__main__":
    rng = np.random.default_rng(0)
    x = rng.standard_normal((B, S, D), dtype=np.float32)
    W = rng.standard_normal((O, D), dtype=np.float32) * 0.03
    b = np.zeros((O,), dtype=np.float32)
    y = kernel(x, W, b)
    print("kernel ran, y shape", y.shape, "mean|y|", np.abs(y).mean())


# revision 14
# speedup vs baseline: 1.2427x; 1.2427x over previous
"""BitLinear forward kernel for Trainium2 (8 NeuronCores, data-parallel).

Computes y = sign(x) @ (alpha * code)^T + b where code/alpha are the
per-row ternarization of W (BitNet-style, delta_w = 0.05, delta_a = 0.0).

Sharding: x is split over batch*seq (16384 rows) across 8 cores; W is
replicated (each core quantizes the full W on-device); outputs are
concatenated on the host.

The matmul runs in fp8 DoubleRow mode (operand values are exactly
{-1, 0, +1}; two d-tiles are paired per PE pass) with fp32 PSUM
accumulation, so integer counts are exact; the per-output-feature alpha
scale is applied in fp32 on eviction. b from setup_inputs() is zeros; a
nonzero b takes a second elementwise pass.

Layout: the contraction dim (d) must live on SBUF partitions for both
matmul operands, so sign(x) and code are bounced through DRAM (bf16) in
half-row blocks and read back through the DMA xbar transpose, then cast
(one wide DVE op each) into paired-d-tile fp8 operands [128, 2, 1024].
Half-width PSUM groups decouple the low o-banks from the second half of
the W pipeline so the PE can start early.

alpha identity: sum(aWc * (aWc>=thr)) == sum(relu(aWc-thr)) + thr*count,
so the numerator comes from an ACT Relu pass with accumulate.
"""

import sys

for _p in ("/opt/trn_rl_repo", "/opt/trn_rl_repo/concourse"):
    if _p not in sys.path:
        sys.path.insert(0, _p)

import numpy as np

import concourse.bass as bass
import concourse.tile as tile
import concourse.mybir as mybir
from concourse import bacc
from concourse.bass_utils import run_bass_kernel_spmd

# Problem shape (hardcoded per contract)
B, S, D, O = 4, 4096, 2048, 2048
N_CORES = 8
T = (B * S) // N_CORES  # 2048 token rows per core
DELTA_W = 0.05

P = 128
TT = T // P   # 16 t-tiles
DT = D // P   # 16 d-tiles
DP = DT // 2  # 8 d-pair tiles (DoubleRow)
WT = O // P   # 16 W row-tiles
NB = 4        # psum banks per t-tile (512 f32 each)
NBW = O // NB  # 512
H = 2         # half blocks
HR = T // H   # 1024 rows per half

F32 = mybir.dt.float32
BF16 = mybir.dt.bfloat16
FP8 = mybir.dt.float8e4
U16 = mybir.dt.uint16

_CACHE = {}


def _build(with_bias: bool):
    nc = bacc.Bacc("TRN2", target_bir_lowering=False, debug=False,
                   num_devices=N_CORES)
    x_d = nc.dram_tensor("x", [T, D], F32, kind="ExternalInput").ap()
    w_d = nc.dram_tensor("W", [O, D], F32, kind="ExternalInput").ap()
    y_d = nc.dram_tensor("y", [T, O], F32, kind="ExternalOutput").ap()
    if with_bias:
        b_d = nc.dram_tensor("b", [O], F32, kind="ExternalInput").ap()

    with tile.TileContext(nc) as tc:
        with (
            tc.tile_pool(name="dram", bufs=8, space="DRAM") as dram,
            tc.tile_pool(name="wload", bufs=3) as wload,
            tc.tile_pool(name="awc", bufs=3) as awc_pool,
            tc.tile_pool(name="junk", bufs=1) as junk_pool,
            tc.tile_pool(name="wsmall", bufs=2) as wsmall,
            tc.tile_pool(name="stats", bufs=1) as stats,
            tc.tile_pool(name="xload", bufs=2) as xload,
            tc.tile_pool(name="xsign", bufs=2) as xsign,
            tc.tile_pool(name="tpose", bufs=2) as tpose,
            tc.tile_pool(name="codeT", bufs=DP * H) as codeT_pool,
            tc.tile_pool(name="xqT", bufs=DP * H) as xqT_pool,
            tc.tile_pool(name="psum", bufs=4, space="PSUM") as psum_pool,
            tc.tile_pool(name="yout", bufs=3) as yout,
            tc.tile_pool(name="bcast", bufs=1) as bcast,
        ):
            # half-row DRAM bounce tiles
            xq_dram = [dram.tile([HR, D], BF16, tag=f"xqd{h}",
                                 name=f"xq_dram{h}") for h in range(H)]
            code_dram = [dram.tile([HR, D], BF16, tag=f"cdd{h}",
                                   name=f"code_dram{h}") for h in range(H)]
            alpha_dram = dram.tile([O], F32, tag="alphad")

            # Per-row stats, one column per W row-tile
            S_all = stats.tile([P, WT], F32, tag="S")
            T_all = stats.tile([P, WT], F32, tag="T")
            den_all = stats.tile([P, WT], F32, tag="den")
            relu_all = stats.tile([P, WT], F32, tag="relu")
            negmean_all = stats.tile([P, WT], F32, tag="negmean")
            thr_all = stats.tile([P, WT], F32, tag="thr")
            negthr_all = stats.tile([P, WT], F32, tag="negthr")
            alpha_all = stats.tile([P, WT], F32, tag="alpha")

            act_junk = junk_pool.tile([P, D], F32, tag="act_junk")

            # big pair tiles [128, 2*1024] fp8:
            #   codeT8[dp][h]: halves = d-tiles (2dp, 2dp+1); free covers
            #   o in [h*1024, (h+1)*1024) (banks 2h, 2h+1)
            #   xqT8[dp][h]: free covers t in [h*1024, (h+1)*1024)
            codeT8 = [[codeT_pool.tile([P, 2 * HR], FP8, tag="codeT",
                                       name=f"codeT{dp}_{h}")
                       for h in range(H)] for dp in range(DP)]
            xqT8 = [[xqT_pool.tile([P, 2 * HR], FP8, tag="xqT",
                                   name=f"xqT{dp}_{h}")
                     for h in range(H)] for dp in range(DP)]

            for h in range(H):
                # ---- half h of the W pipeline (wi = 8h..8h+7) -----------
                for j in range(WT // H):
                    wi = h * (WT // H) + j
                    wt = wload.tile([P, D], F32)
                    nc.gpsimd.dma_start(wt[:], w_d[wi * P:(wi + 1) * P, :])
                    # S = sum(W) via ACT Copy with accumulate
                    nc.scalar.activation(
                        out=act_junk[:], in_=wt[:],
                        func=mybir.ActivationFunctionType.Copy,
                        accum_out=S_all[:, wi:wi + 1],
                    )
                    nc.vector.tensor_scalar_mul(
                        negmean_all[:, wi:wi + 1], S_all[:, wi:wi + 1],
                        -1.0 / D,
                    )
                    # aWc = |W - mean|, T = sum(aWc)
                    aWc = awc_pool.tile([P, D], F32)
                    nc.scalar.activation(
                        out=aWc[:], in_=wt[:],
                        func=mybir.ActivationFunctionType.Abs,
                        bias=negmean_all[:, wi:wi + 1],
                        accum_out=T_all[:, wi:wi + 1],
                    )
                    nc.vector.tensor_scalar_mul(
                        thr_all[:, wi:wi + 1], T_all[:, wi:wi + 1],
                        DELTA_W / D,
                    )
                    nc.vector.tensor_scalar_mul(
                        negthr_all[:, wi:wi + 1], T_all[:, wi:wi + 1],
                        -DELTA_W / D,
                    )
                    # sgn = Sign(W - mean) bf16
                    sgn = wsmall.tile([P, D], BF16, tag="sgn")
                    nc.scalar.activation(
                        out=sgn[:], in_=wt[:],
                        func=mybir.ActivationFunctionType.Sign,
                        bias=negmean_all[:, wi:wi + 1],
                    )
                    # R = sum(relu(aWc - thr))  (alpha numerator part)
                    nc.scalar.activation(
                        out=act_junk[:], in_=aWc[:],
                        func=mybir.ActivationFunctionType.Relu,
                        bias=negthr_all[:, wi:wi + 1],
                        accum_out=relu_all[:, wi:wi + 1],
                    )
                    # s01 = (aWc >= thr), den = count
                    s01 = wsmall.tile([P, D], BF16, tag="s01")
                    nc.vector.tensor_scalar(
                        out=s01[:], in0=aWc[:],
                        scalar1=thr_all[:, wi:wi + 1], scalar2=0.0,
                        op0=mybir.AluOpType.is_ge,
                        op1=mybir.AluOpType.add,
                        accum_out=den_all[:, wi:wi + 1],
                    )
                    # code = sgn * s01 (values exactly -1/0/1)
                    code = wsmall.tile([P, D], BF16, tag="code")
                    nc.vector.tensor_mul(code[:], sgn[:], s01[:])
                    nc.scalar.dma_start(
                        code_dram[h][j * P:(j + 1) * P, :], code[:])

                    # ---- same-index t-tile of the x pipeline ------------
                    ti = h * (TT // H) + j
                    xb = xload.tile([P, D], BF16)
                    nc.gpsimd.dma_start(
                        xb[:], x_d[ti * P:(ti + 1) * P, :])  # f32->bf16
                    xq = xsign.tile([P, D], BF16)
                    nc.vector.tensor_scalar(
                        out=xq.bitcast(U16)[:], in0=xb.bitcast(U16)[:],
                        scalar1=0x8000, scalar2=0x3F80,
                        op0=mybir.AluOpType.bitwise_and,
                        op1=mybir.AluOpType.bitwise_or,
                    )
                    nc.scalar.dma_start(
                        xq_dram[h][j * P:(j + 1) * P, :], xq[:])

                # ---- half h transposed reads + fp8 pair converts --------
                for di in range(DT):
                    dp, hd = divmod(di, 2)
                    tb = tpose.tile([P, HR], BF16, tag="tp_code")
                    nc.sync.dma_start_transpose(
                        tb[:], code_dram[h][:, di * P:(di + 1) * P])
                    nc.vector.tensor_copy(
                        out=codeT8[dp][h][:, hd * HR:(hd + 1) * HR],
                        in_=tb[:])
                    tb2 = tpose.tile([P, HR], BF16, tag="tp_xq")
                    nc.sync.dma_start_transpose(
                        tb2[:], xq_dram[h][:, di * P:(di + 1) * P])
                    nc.vector.tensor_copy(
                        out=xqT8[dp][h][:, hd * HR:(hd + 1) * HR],
                        in_=tb2[:])

            # ------ alpha = (R + thr*den) / max(den, 1) ------------------
            num = stats.tile([P, WT], F32, tag="num")
            nc.vector.tensor_mul(num[:], thr_all[:], den_all[:])
            nc.vector.tensor_add(num[:], num[:], relu_all[:])
            denc = stats.tile([P, WT], F32, tag="denc")
            nc.vector.tensor_scalar_max(denc[:], den_all[:], 1.0)
            rden = stats.tile([P, WT], F32, tag="rden")
            nc.vector.reciprocal(rden[:], denc[:])
            nc.vector.tensor_mul(alpha_all[:], num[:], rden[:])
            nc.sync.dma_start(
                alpha_dram.rearrange("(w p) -> p w", p=P)[:, :], alpha_all[:])
            alphaB = bcast.tile([P, O], F32, tag="alphaB")
            nc.gpsimd.dma_start(
                alphaB[:], alpha_dram.unsqueeze(0).to_broadcast((P, O)))
            if with_bias:
                biasB = bcast.tile([P, O], F32, tag="biasB")
                nc.gpsimd.dma_start(
                    biasB[:], b_d.unsqueeze(0).to_broadcast((P, O)))

            # -------- main matmul (DoubleRow fp8, half-width groups) -----
            def mm_group(ti, oh):
                """Accumulate y[ti-block, oh*1024:(oh+1)*1024]."""
                ps = psum_pool.tile([P, O // H], F32, tag="ps", name=f"ps{ti}_{oh}")
                q, r = divmod(ti, TT // H)
                for dp in range(DP):
                    lhsT = xqT8[dp][q].rearrange(
                        "p (two m) -> p two m", two=2)[:, :, r * P:(r + 1) * P]
                    rhs_all = codeT8[dp][oh].rearrange(
                        "p (two n) -> p two n", two=2)
                    for bk in range(NB // H):
                        nc.tensor.matmul(
                            ps[:, bk * NBW:(bk + 1) * NBW],
                            lhsT,
                            rhs_all[:, :, bk * NBW:(bk + 1) * NBW],
                            start=(dp == 0),
                            stop=(dp == DP - 1),
                            perf_mode=mybir.MatmulPerfMode.DoubleRow,
                        )
                ysb = yout.tile([P, O // H], F32, tag="ysb")
                nc.vector.tensor_mul(
                    ysb[:], ps[:], alphaB[:, oh * (O // H):(oh + 1) * (O // H)])
                if with_bias:
                    nc.vector.tensor_add(
                        ysb[:], ysb[:],
                        biasB[:, oh * (O // H):(oh + 1) * (O // H)])
                nc.scalar.dma_start(
                    y_d[ti * P:(ti + 1) * P,
                        oh * (O // H):(oh + 1) * (O // H)], ysb[:])

            # low o-half for all t-tiles first (only needs W-half 0),
            # then the high o-half.
            for oh in range(H):
                for ti in range(TT):
                    mm_group(ti, oh)

    nc.compile()
    return nc


def _get_nc(with_bias: bool):
    key = with_bias
    if key not in _CACHE:
        _CACHE[key] = _build(with_bias)
    return _CACHE[key]


def kernel(x: np.ndarray, W: np.ndarray, b: np.ndarray) -> np.ndarray:
    x = np.asarray(x, dtype=np.float32)
    W = np.ascontiguousarray(W, dtype=np.float32)
    b = np.asarray(b, dtype=np.float32)
    with_bias = bool(np.any(b))

    nc = _get_nc(with_bias)

    xf = np.ascontiguousarray(x.reshape(B * S, D))
    in_maps = []
    for c in range(N_CORES):
        m = {"x": np.ascontiguousarray(xf[c * T:(c + 1) * T]), "W": W}
        if with_bias:
            m["b"] = b
        in_maps.append(m)

    res = run_bass_kernel_spmd(nc, in_maps, core_ids=list(range(N_CORES)))
    y = np.concatenate([res.results[c]["y"] for c in range(N_CORES)], axis=0)
    return np.ascontiguousarray(y.reshape(B, S, O))


if __name__ == "__main__":
    rng = np.random.default_rng(0)
    x = rng.standard_normal((B, S, D), dtype=np.float32)
    W = rng.standard_normal((O, D), dtype=np.float32) * 0.03
    b = np.zeros((O,), dtype=np.float32)
    y = kernel(x, W, b)
    print("kernel ran, y shape", y.shape, "mean|y|", np.abs(y).mean())
